# revision 42
# baseline (speedup 1.0000x reference)
"""Trainium2 Bass/Tile kernel: single-head attention (B=8, S=2048, E=1024, DQ=DV=128).

Data-parallel over the batch: one batch element per NeuronCore (8 cores), no
collectives. Host pre-transposes activations into stripe-blocked bf16 so the
contraction dim lands on SBUF partitions and every DMA is a contiguous 512KB
block with 4KB partition lines; everything else runs on-chip:

  qT/kT/vT = W.T @ xT          (PE, bf16 in / fp32 PSUM accum, bias added on DVE copy)
  v_aug    = transpose(vT) ++ ones column   (PE transpose; ones column makes the
                                             AV matmul emit softmax row sums for free)
  scoresT  = kT_chunk.T @ qT   ([keys, queries] layout; causal upper blocks skipped)
  attnT    = exp(scoresT/sqrt(DQ) + pad_bias)  (ACT; pad mask is a per-partition bias;
                                               no max-subtraction needed: |scores| < ~3)
  out[q,:] = (attnT.T @ v_aug)[:, :DV] * recip(row_sum)   (PE + DVE recip/scale)

Schedule (v-first column pipeline): v0/v1 stream first so v-proj, the PE
transposes, and the v_aug packing complete while q/k are still in flight; the
(q,k) pairs follow as [E, 512]-column stripes split 512KB-half per HWDGE ring
(~420GB/s aggregate, 4KB partition lines), with v2/v3 at the sync ring's tail.
After pair p lands, both stripes are projected and the scores column p runs
for every ready key block, feeding the serialized ~22us exp chain on the
scalar engine.  The scalar ring deliberately carries only ~5MB of input: a
dma_start blocks its engine until the ring drains, so a lighter ring frees
the scalar engine for exps sooner.  Each AV output tile i only needs scores
column i//4, so the AV chains for column group g-1 (split to one emitter per
matmul) are WOVEN between the scores pieces of column g — the PE fills the
exp-drain stalls with AV work and the tail collapses to the last column's
exps plus one AV group.  Elementwise drains are spread across engines (DVE:
PSUM drains, v_aug packing, causal tri mask, normalize; scalar: exp) and the
AV PSUM ring is 4 deep so the normalize never stalls the chains.  No dummy
warm-up matmuls: sustained PE activity triggers the chip's power throttle
(throttle_avg_util_limit ~0.63-0.76), so idle PE slots are cheaper than
synthetic work.
"""

import numpy as np
import ml_dtypes
from contextlib import ExitStack

B, S, E, DQ, DV = 8, 2048, 1024, 128, 128
EC = E // 128    # contraction chunks
SC = S // 128    # sequence chunks
QB = 512         # stripe / matmul moving-dim width
NG = S // QB     # stripe granules per tensor
RSQRT_DQ = 1.0 / float(np.sqrt(DQ))
NEG = np.float32(-1e9)
_BF16 = ml_dtypes.bfloat16

_prog = None


def _build_program():
    import concourse.bacc as bacc
    import concourse.mybir as mybir
    import concourse.tile as tile

    f32 = mybir.dt.float32
    bf16 = mybir.dt.bfloat16
    AF = mybir.ActivationFunctionType
    ALU = mybir.AluOpType

    nc = bacc.Bacc("TRN2", target_bir_lowering=False, debug=False)

    # stripe-blocked activations, split into ring halves: [granule,
    # partition, rowblock, col] so each half-granule is one contiguous 512KB
    # DMA with 4KB partition lines (the fastest measured DGE shape).
    d_x = {n + h: nc.dram_tensor(n + h, [NG, 128, 4, QB], bf16,
                                 kind="ExternalInput").ap()
           for n in ("qT", "kT", "vT") for h in ("S", "C")}
    d_w = {n: nc.dram_tensor(n, [128, EC, 128], bf16, kind="ExternalInput").ap()
           for n in ("wq", "wk", "wv")}
    d_b = {n: nc.dram_tensor(n, [128, 1], f32, kind="ExternalInput").ap()
           for n in ("bq", "bk", "bv")}
    d_padb = nc.dram_tensor("padb", [128, SC], f32, kind="ExternalInput").ap()
    d_tri = nc.dram_tensor("tri", [128, 128], bf16, kind="ExternalInput").ap()
    d_eye = nc.dram_tensor("eye", [128, 128], bf16, kind="ExternalInput").ap()
    d_out = nc.dram_tensor("out", [S, DV], f32, kind="ExternalOutput").ap()

    with tile.TileContext(nc) as tc, ExitStack() as ctx:
        consts = ctx.enter_context(tc.tile_pool(name="consts", bufs=1))
        xin_p = ctx.enter_context(tc.tile_pool(name="xin", bufs=1))
        proj_p = ctx.enter_context(tc.tile_pool(name="proj", bufs=1))
        attn_p = ctx.enter_context(tc.tile_pool(name="attn", bufs=1))
        out_p = ctx.enter_context(tc.tile_pool(name="outp", bufs=4))
        # PSUM: proj ring 2 banks + scores/vtrans ring 2 + AV/dummy ring 4 = 8
        ps_p = ctx.enter_context(tc.tile_pool(name="ps_p", bufs=2, space="PSUM"))
        ps_sc = ctx.enter_context(tc.tile_pool(name="ps_sc", bufs=2, space="PSUM"))
        ps_av = ctx.enter_context(tc.tile_pool(name="ps_av", bufs=4, space="PSUM"))

        # One-time exp LUT load: scalar engine's first instruction.
        warm = consts.tile([128, 1], f32, tag="warm")
        nc.vector.memset(warm[:, :], 0.0)
        wo = consts.tile([128, 1], f32, tag="warmo")
        nc.scalar.activation(wo[:, :], warm[:, :], AF.Exp)

        # --- input stripes ------------------------------------------------
        xg = {"qT": [[None, None] for _ in range(NG)],
              "kT": [[None, None] for _ in range(NG)],
              "vT": [[None, None] for _ in range(NG)]}

        def half_dma(name, g, h, eng):
            t = xin_p.tile([128, 4, QB], bf16, tag=f"x{name[0]}{g}h{h}",
                           name=f"x{name[0]}{g}h{h}")
            eng.dma_start(t[:, :, :], d_x[name + h][g])
            xg[name][g]["SC".index(h)] = t

        def stripe_dma(name, g):
            half_dma(name, g, "S", nc.sync)
            half_dma(name, g, "C", nc.scalar)

        def xchunk(name, g, c):
            return xg[name][g][c // 4][:, c % 4, :]

        w_sb = {}
        b_sb = {}

        def w_dma(eng, n):
            t = consts.tile([128, EC, 128], bf16, tag="w" + n, name="w" + n)
            eng.dma_start(t[:, :, :], d_w["w" + n])
            w_sb["w" + n] = t
            t = consts.tile([128, 1], f32, tag="b" + n, name="b" + n)
            eng.dma_start(t[:, :], d_b["b" + n])
            b_sb["b" + n] = t

        # Two HW rings only (a third gpsimd SWDGE queue collapses aggregate
        # HBM throughput).  The scalar ring's drain time sets when its engine
        # is free to start the exp chain, so its share stops at k3; the
        # latest-needed v2/v3 halves all ride the sync ring instead.
        stripe_dma("vT", 0)
        w_dma(nc.sync, "v")
        w_dma(nc.scalar, "k")
        eye = consts.tile([128, 128], bf16, tag="eye")
        nc.scalar.dma_start(eye[:, :], d_eye)
        stripe_dma("vT", 1)
        w_dma(nc.sync, "q")
        tri = consts.tile([128, 128], bf16, tag="tri")
        nc.scalar.dma_start(tri[:, :], d_tri)
        padb = consts.tile([128, SC], f32, tag="padb")
        nc.scalar.dma_start(padb[:, :], d_padb)
        stripe_dma("qT", 0)
        stripe_dma("kT", 0)
        stripe_dma("qT", 1)
        stripe_dma("kT", 1)
        stripe_dma("qT", 2)
        stripe_dma("kT", 2)
        stripe_dma("qT", 3)
        stripe_dma("kT", 3)
        half_dma("vT", 2, "S", nc.sync)
        half_dma("vT", 2, "C", nc.sync)
        half_dma("vT", 3, "S", nc.sync)
        half_dma("vT", 3, "C", nc.sync)

        qT = proj_p.tile([128, S], bf16, tag="qT")
        kT = proj_p.tile([128, S], bf16, tag="kT")
        vT = proj_p.tile([128, S], bf16, tag="vT")

        def proj_stripe(name, g, dst):
            ps = ps_p.tile([128, QB], f32, tag="pp", name=f"pp_{name[0]}{g}")
            w = w_sb["w" + name[0]]
            for c in range(EC):
                nc.tensor.matmul(ps[:, :], w[:, c, :], xchunk(name, g, c),
                                 start=(c == 0), stop=(c == EC - 1))
            nc.vector.tensor_scalar(dst[:, g * QB:(g + 1) * QB], ps[:, :],
                                    b_sb["b" + name[0]][:, :], None, ALU.add)

        attnT = [attn_p.tile([128, S - j * 128], bf16, tag=f"attnT{j}",
                             name=f"attnT{j}")
                 for j in range(SC)]

        def scores_piece(j, p):
            # scoresT[j], query columns [p*QB, (p+1)*QB) -> exp -> attnT[j]
            q0 = max(p * QB, j * 128)
            m = (p + 1) * QB - q0
            ps = ps_sc.tile([128, QB], f32, tag="sc", name=f"sc{j}_{p}")
            nc.tensor.matmul(ps[:, 0:m], kT[:, j * 128:(j + 1) * 128],
                             qT[:, q0:q0 + m], start=True, stop=True)
            a0 = q0 - j * 128
            nc.scalar.activation(attnT[j][:, a0:a0 + m], ps[:, 0:m], AF.Exp,
                                 bias=padb[:, j:j + 1], scale=RSQRT_DQ)
            if p == j // 4:
                # in-block causal mask on the diagonal block (keep k <= q)
                nc.vector.tensor_mul(attnT[j][:, 0:128], attnT[j][:, 0:128],
                                     tri[:, :])

        vaug = [None] * SC

        def av_emitters(i):
            # AV chain for output tile i, one emitter per matmul so the weave
            # can spread them between scores pieces, plus a finalize emitter
            # (softmax normalize on DVE + store on the sync ring).
            ps = ps_av.tile([128, QB], f32, tag="av", name=f"av{i}")

            def mm(j, ps=ps, i=i):
                nc.tensor.matmul(ps[:, 0:DV + 1],
                                 attnT[j][:, (i - j) * 128:(i - j) * 128 + 128],
                                 vaug[j][:, :], start=(j == 0), stop=(j == i))

            def fin(ps=ps, i=i):
                rec = out_p.tile([128, 1], f32, tag="rec")
                nc.vector.reciprocal(rec[:, :], ps[:, DV:DV + 1])
                ot = out_p.tile([128, DV], f32, tag="ot")
                nc.vector.tensor_scalar(ot[:, :], ps[:, 0:DV], rec[:, :], None,
                                        ALU.mult)
                nc.sync.dma_start(d_out[i * 128:(i + 1) * 128, :], ot[:, :])

            return [lambda j=j: mm(j) for j in range(i + 1)] + [fin]

        def av_group(g):
            ems = []
            for i in range(4 * g, 4 * g + 4):
                ems.extend(av_emitters(i))
            return ems

        def weave(pieces, fillers):
            # emit scores pieces with filler work spread between them so the
            # exp drain (one piece / ~560ns) never idles the PE; at most 2
            # fillers per piece so the piece matmuls keep the exp chain fed,
            # leftover fillers run while the last exps drain.
            na, nb = len(pieces), len(fillers)
            bi, acc = 0, 0.0
            r = nb / max(na, 1)
            for a in pieces:
                a()
                acc += r
                while acc >= 1.0 and bi < nb:
                    fillers[bi]()
                    bi += 1
                    acc -= 1.0
            while bi < nb:
                fillers[bi]()
                bi += 1

        def vtrans(j):
            ps = ps_p.tile([128, 128], bf16, tag="pp", name="vt")
            nc.tensor.transpose(ps[:, 0:128], vT[:, j * 128:(j + 1) * 128],
                                eye[:, :])
            va = attn_p.tile([128, DV + 1], bf16, tag=f"vaug{j}")
            nc.vector.tensor_copy(va[:, 0:DV], ps[:, 0:128])
            nc.vector.memset(va[:, DV:DV + 1], 1.0)
            vaug[j] = va

        def vstripe_emitters(g):
            ps = ps_p.tile([128, QB], f32, tag="pp", name=f"pp_v{g}")
            w = w_sb["wv"]

            def mm(c, ps=ps, g=g):
                nc.tensor.matmul(ps[:, :], w[:, c, :], xchunk("vT", g, c),
                                 start=(c == 0), stop=(c == EC - 1))

            def drain(ps=ps, g=g):
                nc.vector.tensor_scalar(vT[:, g * QB:(g + 1) * QB], ps[:, :],
                                        b_sb["bv"][:, :], None, ALU.add)

            return ([lambda c=c: mm(c) for c in range(EC)] + [drain] +
                    [lambda j=j: vtrans(j) for j in range(4 * g, 4 * g + 4)])

        # ---- v phase: stripe 0 projected + transposed up front ----
        for f in vstripe_emitters(0):
            f()

        # ---- (q,k) pairs: scores column p woven with AV group p-1 and the
        # remaining v stripes ----
        for p in range(NG):
            proj_stripe("qT", p, qT)
            proj_stripe("kT", p, kT)
            pieces = [lambda j=j, p=p: scores_piece(j, p)
                      for j in range(4 * p + 4)]
            if p == 0:
                for a in pieces:
                    a()
                for f in vstripe_emitters(1):
                    f()
            elif p == 1:
                weave(pieces, av_group(0))
            elif p == 2:
                weave(pieces, av_group(1))
            else:
                av2 = av_group(2)
                weave(pieces, vstripe_emitters(2) + av2[:20]
                      + vstripe_emitters(3) + av2[20:])

        # ---- last AV group ----
        for f in av_group(3):
            f()

    nc.compile()
    return nc


def _prep_inputs(pad_mask, query, key, value, Wq, bq, Wk, bk, Wv, bv):
    def wprep(w):
        return np.ascontiguousarray(
            np.asarray(w, np.float32).astype(_BF16).reshape(EC, 128, 128)
            .transpose(1, 0, 2))

    def bprep(v):
        return np.ascontiguousarray(np.asarray(v, np.float32).reshape(128, 1))

    shared = {
        "wq": wprep(Wq), "wk": wprep(Wk), "wv": wprep(Wv),
        "bq": bprep(bq), "bk": bprep(bk), "bv": bprep(bv),
        "tri": np.triu(np.ones((128, 128), np.float32)).astype(_BF16),
        "eye": np.eye(128, dtype=np.float32).astype(_BF16),
    }
    pad_mask = np.asarray(pad_mask)
    query = np.asarray(query, np.float32)
    key = np.asarray(key, np.float32)
    value = np.asarray(value, np.float32)

    def xprep(x):
        # x [S, E] -> per-ring stripe blocks [NG, 128, 4, QB]:
        # [g, p, r, s] = x[g*QB+s, h*512 + r*128 + p]
        a = x.reshape(NG, QB, 2, 4, 128)
        return (np.ascontiguousarray(a[:, :, 0].transpose(0, 3, 2, 1)).astype(_BF16),
                np.ascontiguousarray(a[:, :, 1].transpose(0, 3, 2, 1)).astype(_BF16))

    in_maps = []
    for b in range(B):
        padb = np.ascontiguousarray(
            np.where(pad_mask[b], NEG, np.float32(0.0)).reshape(SC, 128).T)
        qS, qC = xprep(query[b])
        kS, kC = xprep(key[b])
        vS, vC = xprep(value[b])
        in_maps.append({
            **shared,
            "qTS": qS, "qTC": qC, "kTS": kS, "kTC": kC, "vTS": vS, "vTC": vC,
            "padb": padb.astype(np.float32),
        })
    return in_maps


def _run(in_maps, trace=False, **kwargs):
    global _prog
    from concourse.bass_utils import run_bass_kernel_spmd
    if _prog is None:
        _prog = _build_program()
    return run_bass_kernel_spmd(_prog, in_maps, list(range(B)), trace=trace,
                                **kwargs)


def kernel(pad_mask, query, key, value, Wq, bq, Wk, bk, Wv, bv):
    in_maps = _prep_inputs(pad_mask, query, key, value, Wq, bq, Wk, bk, Wv, bv)
    res = _run(in_maps)
    out = np.stack([np.asarray(res.results[i]["out"]) for i in range(B)])
    return np.ascontiguousarray(out.astype(np.float32))


# revision 43
# speedup vs baseline: 1.1197x; 1.1197x over previous
"""Trainium2 Bass/Tile kernel: single-head attention (B=8, S=2048, E=1024, DQ=DV=128).

Data-parallel over the batch: one batch element per NeuronCore (8 cores), no
collectives. Host pre-transposes activations into stripe-blocked bf16 so the
contraction dim lands on SBUF partitions and every DMA is a contiguous 512KB
block with 4KB partition lines; everything else runs on-chip:

  qT/kT/vT = W.T @ xT          (PE, bf16 in / fp32 PSUM accum, bias added on DVE copy)
  v_aug    = transpose(vT) ++ ones column   (PE transpose; ones column makes the
                                             AV matmul emit softmax row sums for free)
  scoresT  = kT_chunk.T @ qT   ([keys, queries] layout; causal upper blocks skipped)
  attnT    = exp(scoresT/sqrt(DQ) + pad_bias)  (ACT; pad mask is a per-partition bias;
                                               no max-subtraction needed: |scores| < ~3)
  out[q,:] = (attnT.T @ v_aug)[:, :DV] * recip(row_sum)   (PE + DVE recip/scale)

Schedule (v-first column pipeline): v0/v1 stream first so v-proj, the PE
transposes, and the v_aug packing complete while q/k are still in flight; the
(q,k) pairs follow as [E, 512]-column stripes split 512KB-half per HWDGE ring
(~420GB/s aggregate, 4KB partition lines), with v2/v3 at the sync ring's tail.
After pair p lands, both stripes are projected and the scores column p runs
for every ready key block, feeding the serialized ~22us exp chain on the
scalar engine.  The scalar ring deliberately carries only ~5MB of input: a
dma_start blocks its engine until the ring drains, so a lighter ring frees
the scalar engine for exps sooner.  Each AV output tile i only needs scores
column i//4, so the AV chains for column group g-1 (split to one emitter per
matmul) are WOVEN between the scores pieces of column g — the PE fills the
exp-drain stalls with AV work and the tail collapses to the last column's
exps plus one AV group.  Elementwise drains are spread across engines (DVE:
PSUM drains, v_aug packing, causal tri mask, normalize; scalar: exp) and the
AV PSUM ring is 4 deep so the normalize never stalls the chains.  No dummy
warm-up matmuls: sustained PE activity triggers the chip's power throttle
(throttle_avg_util_limit ~0.63-0.76), so idle PE slots are cheaper than
synthetic work.
"""

import numpy as np
import ml_dtypes
from contextlib import ExitStack

B, S, E, DQ, DV = 8, 2048, 1024, 128, 128
EC = E // 128    # contraction chunks
SC = S // 128    # sequence chunks
QB = 512         # stripe / matmul moving-dim width
NG = S // QB     # stripe granules per tensor
RSQRT_DQ = 1.0 / float(np.sqrt(DQ))
NEG = np.float32(-1e9)
_BF16 = ml_dtypes.bfloat16

_prog = None


def _build_program():
    import concourse.bacc as bacc
    import concourse.mybir as mybir
    import concourse.tile as tile

    f32 = mybir.dt.float32
    bf16 = mybir.dt.bfloat16
    AF = mybir.ActivationFunctionType
    ALU = mybir.AluOpType

    nc = bacc.Bacc("TRN2", target_bir_lowering=False, debug=False)

    # stripe-blocked activations, split into ring halves: [granule,
    # partition, rowblock, col] so each half-granule is one contiguous 512KB
    # DMA with 4KB partition lines (the fastest measured DGE shape).
    d_x = {n + h: nc.dram_tensor(n + h, [NG, 128, 4, QB], bf16,
                                 kind="ExternalInput").ap()
           for n in ("qT", "kT", "vT") for h in ("S", "C")}
    d_w = {n: nc.dram_tensor(n, [128, EC, 128], bf16, kind="ExternalInput").ap()
           for n in ("wq", "wk", "wv")}
    d_b = {n: nc.dram_tensor(n, [128, 1], f32, kind="ExternalInput").ap()
           for n in ("bq", "bk", "bv")}
    d_padb = nc.dram_tensor("padb", [128, SC], f32, kind="ExternalInput").ap()
    d_tri = nc.dram_tensor("tri", [128, 128], bf16, kind="ExternalInput").ap()
    d_eye = nc.dram_tensor("eye", [128, 128], bf16, kind="ExternalInput").ap()
    d_out = nc.dram_tensor("out", [S, DV], f32, kind="ExternalOutput").ap()

    with tile.TileContext(nc) as tc, ExitStack() as ctx:
        consts = ctx.enter_context(tc.tile_pool(name="consts", bufs=1))
        xin_p = ctx.enter_context(tc.tile_pool(name="xin", bufs=1))
        proj_p = ctx.enter_context(tc.tile_pool(name="proj", bufs=1))
        attn_p = ctx.enter_context(tc.tile_pool(name="attn", bufs=1))
        out_p = ctx.enter_context(tc.tile_pool(name="outp", bufs=4))
        # PSUM: proj ring 2 banks + scores/vtrans ring 2 + AV/dummy ring 4 = 8
        ps_p = ctx.enter_context(tc.tile_pool(name="ps_p", bufs=2, space="PSUM"))
        ps_sc = ctx.enter_context(tc.tile_pool(name="ps_sc", bufs=2, space="PSUM"))
        ps_av = ctx.enter_context(tc.tile_pool(name="ps_av", bufs=4, space="PSUM"))

        # One-time exp LUT load: scalar engine's first instruction.
        warm = consts.tile([128, 1], f32, tag="warm")
        nc.vector.memset(warm[:, :], 0.0)
        wo = consts.tile([128, 1], f32, tag="warmo")
        nc.scalar.activation(wo[:, :], warm[:, :], AF.Exp)

        # --- input stripes ------------------------------------------------
        xg = {"qT": [[None, None] for _ in range(NG)],
              "kT": [[None, None] for _ in range(NG)],
              "vT": [[None, None] for _ in range(NG)]}

        def half_dma(name, g, h, eng):
            t = xin_p.tile([128, 4, QB], bf16, tag=f"x{name[0]}{g}h{h}",
                           name=f"x{name[0]}{g}h{h}")
            eng.dma_start(t[:, :, :], d_x[name + h][g])
            xg[name][g]["SC".index(h)] = t

        def stripe_dma(name, g):
            half_dma(name, g, "S", nc.sync)
            half_dma(name, g, "C", nc.scalar)

        def xchunk(name, g, c):
            return xg[name][g][c // 4][:, c % 4, :]

        w_sb = {}
        b_sb = {}

        def w_dma(eng, n):
            t = consts.tile([128, EC, 128], bf16, tag="w" + n, name="w" + n)
            eng.dma_start(t[:, :, :], d_w["w" + n])
            w_sb["w" + n] = t
            t = consts.tile([128, 1], f32, tag="b" + n, name="b" + n)
            eng.dma_start(t[:, :], d_b["b" + n])
            b_sb["b" + n] = t

        # Two HW rings only (a third gpsimd SWDGE queue collapses aggregate
        # HBM throughput).  The scalar ring's drain time sets when its engine
        # is free to start the exp chain, so its share stops at k3; the
        # latest-needed v2/v3 halves all ride the sync ring instead.
        stripe_dma("vT", 0)
        w_dma(nc.sync, "v")
        w_dma(nc.scalar, "k")
        eye = consts.tile([128, 128], bf16, tag="eye")
        nc.scalar.dma_start(eye[:, :], d_eye)
        stripe_dma("vT", 1)
        w_dma(nc.sync, "q")
        tri = consts.tile([128, 128], bf16, tag="tri")
        nc.scalar.dma_start(tri[:, :], d_tri)
        padb = consts.tile([128, SC], f32, tag="padb")
        nc.scalar.dma_start(padb[:, :], d_padb)
        stripe_dma("qT", 0)
        stripe_dma("kT", 0)
        stripe_dma("qT", 1)
        stripe_dma("kT", 1)
        stripe_dma("qT", 2)
        stripe_dma("kT", 2)
        stripe_dma("qT", 3)
        stripe_dma("kT", 3)
        half_dma("vT", 2, "S", nc.sync)
        half_dma("vT", 2, "C", nc.sync)
        half_dma("vT", 3, "S", nc.sync)
        half_dma("vT", 3, "C", nc.sync)

        qT = proj_p.tile([128, S], bf16, tag="qT")
        kT = proj_p.tile([128, S], bf16, tag="kT")
        vT = proj_p.tile([128, S], bf16, tag="vT")

        def proj_stripe(name, g, dst):
            ps = ps_p.tile([128, QB], f32, tag="pp", name=f"pp_{name[0]}{g}")
            w = w_sb["w" + name[0]]
            for c in range(EC):
                nc.tensor.matmul(ps[:, :], w[:, c, :], xchunk(name, g, c),
                                 start=(c == 0), stop=(c == EC - 1))
            nc.vector.tensor_scalar(dst[:, g * QB:(g + 1) * QB], ps[:, :],
                                    b_sb["b" + name[0]][:, :], None, ALU.add)

        attnT = [attn_p.tile([128, S - j * 128], bf16, tag=f"attnT{j}",
                             name=f"attnT{j}")
                 for j in range(SC)]

        def scores_piece(j, p):
            # scoresT[j], query columns [p*QB, (p+1)*QB) -> exp -> attnT[j]
            q0 = max(p * QB, j * 128)
            m = (p + 1) * QB - q0
            ps = ps_sc.tile([128, QB], f32, tag="sc", name=f"sc{j}_{p}")
            nc.tensor.matmul(ps[:, 0:m], kT[:, j * 128:(j + 1) * 128],
                             qT[:, q0:q0 + m], start=True, stop=True)
            a0 = q0 - j * 128
            nc.scalar.activation(attnT[j][:, a0:a0 + m], ps[:, 0:m], AF.Exp,
                                 bias=padb[:, j:j + 1], scale=RSQRT_DQ)
            if p == j // 4:
                # in-block causal mask on the diagonal block (keep k <= q)
                nc.vector.tensor_mul(attnT[j][:, 0:128], attnT[j][:, 0:128],
                                     tri[:, :])

        vaug = [None] * SC

        def av_emitters(i):
            # AV chain for output tile i, one emitter per matmul so the weave
            # can spread them between scores pieces, plus a finalize emitter
            # (softmax normalize on DVE + store on the sync ring).
            ps = ps_av.tile([128, QB], f32, tag="av", name=f"av{i}")

            def mm(j, ps=ps, i=i):
                nc.tensor.matmul(ps[:, 0:DV + 1],
                                 attnT[j][:, (i - j) * 128:(i - j) * 128 + 128],
                                 vaug[j][:, :], start=(j == 0), stop=(j == i))

            def fin(ps=ps, i=i):
                rec = out_p.tile([128, 1], f32, tag="rec")
                nc.vector.reciprocal(rec[:, :], ps[:, DV:DV + 1])
                ot = out_p.tile([128, DV], f32, tag="ot")
                nc.vector.tensor_scalar(ot[:, :], ps[:, 0:DV], rec[:, :], None,
                                        ALU.mult)
                nc.sync.dma_start(d_out[i * 128:(i + 1) * 128, :], ot[:, :])

            return [lambda j=j: mm(j) for j in range(i + 1)] + [fin]

        def av_group(g):
            ems = []
            for i in range(4 * g, 4 * g + 4):
                ems.extend(av_emitters(i))
            return ems

        def weave(pieces, fillers):
            # emit scores pieces with filler work spread between them so the
            # exp drain (one piece / ~560ns) never idles the PE; at most 2
            # fillers per piece so the piece matmuls keep the exp chain fed,
            # leftover fillers run while the last exps drain.
            na, nb = len(pieces), len(fillers)
            bi, acc = 0, 0.0
            r = nb / max(na, 1)
            for a in pieces:
                a()
                acc += r
                while acc >= 1.0 and bi < nb:
                    fillers[bi]()
                    bi += 1
                    acc -= 1.0
            while bi < nb:
                fillers[bi]()
                bi += 1

        def vtrans(j):
            ps = ps_p.tile([128, 128], bf16, tag="pp", name="vt")
            nc.tensor.transpose(ps[:, 0:128], vT[:, j * 128:(j + 1) * 128],
                                eye[:, :])
            va = attn_p.tile([128, DV + 1], bf16, tag=f"vaug{j}")
            nc.vector.tensor_copy(va[:, 0:DV], ps[:, 0:128])
            nc.vector.memset(va[:, DV:DV + 1], 1.0)
            vaug[j] = va

        def vstripe_emitters(g):
            ps = ps_p.tile([128, QB], f32, tag="pp", name=f"pp_v{g}")
            w = w_sb["wv"]

            def mm(c, ps=ps, g=g):
                nc.tensor.matmul(ps[:, :], w[:, c, :], xchunk("vT", g, c),
                                 start=(c == 0), stop=(c == EC - 1))

            def drain(ps=ps, g=g):
                nc.vector.tensor_scalar(vT[:, g * QB:(g + 1) * QB], ps[:, :],
                                        b_sb["bv"][:, :], None, ALU.add)

            return ([lambda c=c: mm(c) for c in range(EC)] + [drain] +
                    [lambda j=j: vtrans(j) for j in range(4 * g, 4 * g + 4)])

        # ---- front-loaded projections: everything whose data lands before
        # the scalar ring frees up (the exp-chain "wall" at ~30us) runs
        # first, so the post-wall segment only carries col0-col3 scores, the
        # AV chains, pair3 and the v2/v3 stripes — balancing the PE against
        # the ~22us exp chain instead of trailing it. ----
        for f in vstripe_emitters(0):
            f()
        proj_stripe("qT", 0, qT)
        proj_stripe("kT", 0, kT)
        for f in vstripe_emitters(1):
            f()
        proj_stripe("qT", 1, qT)
        proj_stripe("kT", 1, kT)
        proj_stripe("qT", 2, qT)
        proj_stripe("kT", 2, kT)

        def col(p):
            return [lambda j=j, p=p: scores_piece(j, p)
                    for j in range(4 * p + 4)]

        for a in col(0):
            a()
        weave(col(1), av_group(0))
        proj_stripe("qT", 3, qT)
        proj_stripe("kT", 3, kT)
        weave(col(2), av_group(1))
        av2 = av_group(2)
        weave(col(3), vstripe_emitters(2) + av2[:20]
              + vstripe_emitters(3) + av2[20:])

        # ---- last AV group ----
        for f in av_group(3):
            f()

    nc.compile()
    return nc


def _prep_inputs(pad_mask, query, key, value, Wq, bq, Wk, bk, Wv, bv):
    def wprep(w):
        return np.ascontiguousarray(
            np.asarray(w, np.float32).astype(_BF16).reshape(EC, 128, 128)
            .transpose(1, 0, 2))

    def bprep(v):
        return np.ascontiguousarray(np.asarray(v, np.float32).reshape(128, 1))

    shared = {
        "wq": wprep(Wq), "wk": wprep(Wk), "wv": wprep(Wv),
        "bq": bprep(bq), "bk": bprep(bk), "bv": bprep(bv),
        "tri": np.triu(np.ones((128, 128), np.float32)).astype(_BF16),
        "eye": np.eye(128, dtype=np.float32).astype(_BF16),
    }
    pad_mask = np.asarray(pad_mask)
    query = np.asarray(query, np.float32)
    key = np.asarray(key, np.float32)
    value = np.asarray(value, np.float32)

    def xprep(x):
        # x [S, E] -> per-ring stripe blocks [NG, 128, 4, QB]:
        # [g, p, r, s] = x[g*QB+s, h*512 + r*128 + p]
        a = x.reshape(NG, QB, 2, 4, 128)
        return (np.ascontiguousarray(a[:, :, 0].transpose(0, 3, 2, 1)).astype(_BF16),
                np.ascontiguousarray(a[:, :, 1].transpose(0, 3, 2, 1)).astype(_BF16))

    in_maps = []
    for b in range(B):
        padb = np.ascontiguousarray(
            np.where(pad_mask[b], NEG, np.float32(0.0)).reshape(SC, 128).T)
        qS, qC = xprep(query[b])
        kS, kC = xprep(key[b])
        vS, vC = xprep(value[b])
        in_maps.append({
            **shared,
            "qTS": qS, "qTC": qC, "kTS": kS, "kTC": kC, "vTS": vS, "vTC": vC,
            "padb": padb.astype(np.float32),
        })
    return in_maps


def _run(in_maps, trace=False, **kwargs):
    global _prog
    from concourse.bass_utils import run_bass_kernel_spmd
    if _prog is None:
        _prog = _build_program()
    return run_bass_kernel_spmd(_prog, in_maps, list(range(B)), trace=trace,
                                **kwargs)


def kernel(pad_mask, query, key, value, Wq, bq, Wk, bk, Wv, bv):
    in_maps = _prep_inputs(pad_mask, query, key, value, Wq, bq, Wk, bk, Wv, bv)
    res = _run(in_maps)
    out = np.stack([np.asarray(res.results[i]["out"]) for i in range(B)])
    return np.ascontiguousarray(out.astype(np.float32))


# revision 44
# speedup vs baseline: 1.1566x; 1.0330x over previous
"""Trainium2 Bass/Tile kernel: single-head attention (B=8, S=2048, E=1024, DQ=DV=128).

Data-parallel over the batch: one batch element per NeuronCore (8 cores), no
collectives. Host pre-transposes activations into stripe-blocked bf16 so the
contraction dim lands on SBUF partitions and every DMA is a contiguous 512KB
block with 4KB partition lines; everything else runs on-chip:

  qT/kT/vT = W.T @ xT          (PE, bf16 in / fp32 PSUM accum, bias added on DVE copy)
  v_aug    = transpose(vT) ++ ones column   (PE transpose; ones column makes the
                                             AV matmul emit softmax row sums for free)
  scoresT  = kT_chunk.T @ qT   ([keys, queries] layout; causal upper blocks skipped)
  attnT    = exp(scoresT/sqrt(DQ) + pad_bias)  (ACT; pad mask is a per-partition bias;
                                               no max-subtraction needed: |scores| < ~3)
  out[q,:] = (attnT.T @ v_aug)[:, :DV] * recip(row_sum)   (PE + DVE recip/scale)

Schedule (v-first column pipeline): v0/v1 stream first so v-proj, the PE
transposes, and the v_aug packing complete while q/k are still in flight; the
(q,k) pairs follow as [E, 512]-column stripes split 512KB-half per HWDGE ring
(~420GB/s aggregate, 4KB partition lines), with v2/v3 at the sync ring's tail.
After pair p lands, both stripes are projected and the scores column p runs
for every ready key block, feeding the serialized ~22us exp chain on the
scalar engine.  The scalar ring deliberately carries only ~5MB of input: a
dma_start blocks its engine until the ring drains, so a lighter ring frees
the scalar engine for exps sooner.  Each AV output tile i only needs scores
column i//4, so the AV chains for column group g-1 (split to one emitter per
matmul) are WOVEN between the scores pieces of column g — the PE fills the
exp-drain stalls with AV work and the tail collapses to the last column's
exps plus one AV group.  Elementwise drains are spread across engines (DVE:
PSUM drains, v_aug packing, causal tri mask, normalize; scalar: exp) and the
AV PSUM ring is 4 deep so the normalize never stalls the chains.  No dummy
warm-up matmuls: sustained PE activity triggers the chip's power throttle
(throttle_avg_util_limit ~0.63-0.76), so idle PE slots are cheaper than
synthetic work.
"""

import numpy as np
import ml_dtypes
from contextlib import ExitStack

B, S, E, DQ, DV = 8, 2048, 1024, 128, 128
EC = E // 128    # contraction chunks
SC = S // 128    # sequence chunks
QB = 512         # stripe / matmul moving-dim width
NG = S // QB     # stripe granules per tensor
RSQRT_DQ = 1.0 / float(np.sqrt(DQ))
NEG = np.float32(-1e9)
_BF16 = ml_dtypes.bfloat16

_prog = None


def _build_program():
    import concourse.bacc as bacc
    import concourse.mybir as mybir
    import concourse.tile as tile

    f32 = mybir.dt.float32
    bf16 = mybir.dt.bfloat16
    AF = mybir.ActivationFunctionType
    ALU = mybir.AluOpType

    nc = bacc.Bacc("TRN2", target_bir_lowering=False, debug=False)

    # stripe-blocked activations, split into ring halves: [granule,
    # partition, rowblock, col] so each half-granule is one contiguous 512KB
    # DMA with 4KB partition lines (the fastest measured DGE shape).
    d_x = {n + h: nc.dram_tensor(n + h, [NG, 128, 4, QB], bf16,
                                 kind="ExternalInput").ap()
           for n in ("qT", "kT", "vT") for h in ("S", "C")}
    d_w = {n: nc.dram_tensor(n, [128, EC, 128], bf16, kind="ExternalInput").ap()
           for n in ("wq", "wk", "wv")}
    d_b = {n: nc.dram_tensor(n, [128, 1], f32, kind="ExternalInput").ap()
           for n in ("bq", "bk", "bv")}
    d_padb = nc.dram_tensor("padb", [128, SC], f32, kind="ExternalInput").ap()
    d_tri = nc.dram_tensor("tri", [128, 128], bf16, kind="ExternalInput").ap()
    d_eye = nc.dram_tensor("eye", [128, 128], bf16, kind="ExternalInput").ap()
    d_out = nc.dram_tensor("out", [S, DV], f32, kind="ExternalOutput").ap()

    with tile.TileContext(nc) as tc, ExitStack() as ctx:
        consts = ctx.enter_context(tc.tile_pool(name="consts", bufs=1))
        xin_p = ctx.enter_context(tc.tile_pool(name="xin", bufs=1))
        proj_p = ctx.enter_context(tc.tile_pool(name="proj", bufs=1))
        attn_p = ctx.enter_context(tc.tile_pool(name="attn", bufs=1))
        out_p = ctx.enter_context(tc.tile_pool(name="outp", bufs=4))
        # PSUM: proj ring 2 banks + scores/vtrans ring 2 + AV/dummy ring 4 = 8
        ps_p = ctx.enter_context(tc.tile_pool(name="ps_p", bufs=2, space="PSUM"))
        ps_sc = ctx.enter_context(tc.tile_pool(name="ps_sc", bufs=2, space="PSUM"))
        ps_av = ctx.enter_context(tc.tile_pool(name="ps_av", bufs=4, space="PSUM"))

        # One-time exp LUT load: scalar engine's first instruction.
        warm = consts.tile([128, 1], f32, tag="warm")
        nc.vector.memset(warm[:, :], 0.0)
        wo = consts.tile([128, 1], f32, tag="warmo")
        nc.scalar.activation(wo[:, :], warm[:, :], AF.Exp)

        # --- input stripes ------------------------------------------------
        xg = {"qT": [[None, None] for _ in range(NG)],
              "kT": [[None, None] for _ in range(NG)],
              "vT": [[None, None] for _ in range(NG)]}

        def half_dma(name, g, h, eng):
            t = xin_p.tile([128, 4, QB], bf16, tag=f"x{name[0]}{g}h{h}",
                           name=f"x{name[0]}{g}h{h}")
            eng.dma_start(t[:, :, :], d_x[name + h][g])
            xg[name][g]["SC".index(h)] = t

        def stripe_dma(name, g):
            half_dma(name, g, "S", nc.sync)
            half_dma(name, g, "C", nc.scalar)

        def xchunk(name, g, c):
            return xg[name][g][c // 4][:, c % 4, :]

        w_sb = {}
        b_sb = {}

        def w_dma(eng, n):
            t = consts.tile([128, EC, 128], bf16, tag="w" + n, name="w" + n)
            eng.dma_start(t[:, :, :], d_w["w" + n])
            w_sb["w" + n] = t
            t = consts.tile([128, 1], f32, tag="b" + n, name="b" + n)
            eng.dma_start(t[:, :], d_b["b" + n])
            b_sb["b" + n] = t

        # Two HW rings only (a third gpsimd SWDGE queue collapses aggregate
        # HBM throughput).  The scalar ring's drain time sets when its engine
        # is free to start the exp chain, so its share stops at k3; the
        # latest-needed v2/v3 halves all ride the sync ring instead.
        stripe_dma("vT", 0)
        w_dma(nc.sync, "v")
        w_dma(nc.scalar, "k")
        eye = consts.tile([128, 128], bf16, tag="eye")
        nc.scalar.dma_start(eye[:, :], d_eye)
        stripe_dma("vT", 1)
        w_dma(nc.sync, "q")
        tri = consts.tile([128, 128], bf16, tag="tri")
        nc.scalar.dma_start(tri[:, :], d_tri)
        padb = consts.tile([128, SC], f32, tag="padb")
        nc.scalar.dma_start(padb[:, :], d_padb)
        stripe_dma("qT", 0)
        stripe_dma("kT", 0)
        stripe_dma("qT", 1)
        stripe_dma("kT", 1)
        stripe_dma("qT", 2)
        stripe_dma("kT", 2)
        stripe_dma("qT", 3)
        stripe_dma("kT", 3)
        half_dma("vT", 2, "S", nc.sync)
        half_dma("vT", 2, "C", nc.sync)
        half_dma("vT", 3, "S", nc.sync)
        half_dma("vT", 3, "C", nc.sync)

        qT = proj_p.tile([128, S], bf16, tag="qT")
        kT = proj_p.tile([128, S], bf16, tag="kT")
        vT = proj_p.tile([128, S], bf16, tag="vT")

        def proj_stripe(name, g, dst):
            ps = ps_p.tile([128, QB], f32, tag="pp", name=f"pp_{name[0]}{g}")
            w = w_sb["w" + name[0]]
            for c in range(EC):
                nc.tensor.matmul(ps[:, :], w[:, c, :], xchunk(name, g, c),
                                 start=(c == 0), stop=(c == EC - 1))
            nc.vector.tensor_scalar(dst[:, g * QB:(g + 1) * QB], ps[:, :],
                                    b_sb["b" + name[0]][:, :], None, ALU.add)

        attnT = [attn_p.tile([128, S - j * 128], bf16, tag=f"attnT{j}",
                             name=f"attnT{j}")
                 for j in range(SC)]

        def scores_piece(j, p):
            # scoresT[j], query columns [p*QB, (p+1)*QB) -> exp -> attnT[j]
            q0 = max(p * QB, j * 128)
            m = (p + 1) * QB - q0
            ps = ps_sc.tile([128, QB], f32, tag="sc", name=f"sc{j}_{p}")
            nc.tensor.matmul(ps[:, 0:m], kT[:, j * 128:(j + 1) * 128],
                             qT[:, q0:q0 + m], start=True, stop=True)
            a0 = q0 - j * 128
            nc.scalar.activation(attnT[j][:, a0:a0 + m], ps[:, 0:m], AF.Exp,
                                 bias=padb[:, j:j + 1], scale=RSQRT_DQ)
            if p == j // 4:
                # in-block causal mask on the diagonal block (keep k <= q).
                # On GPSIMD: it waits on this block's exp, and on the in-order
                # DVE queue that wait would wall every later PSUM drain (and
                # through the PSUM-ring WARs, the pre-wall projections).
                nc.gpsimd.tensor_mul(attnT[j][:, 0:128], attnT[j][:, 0:128],
                                     tri[:, :])

        vaug = [None] * SC

        def av_emitters(i):
            # AV chain for output tile i, one emitter per matmul so the weave
            # can spread them between scores pieces, plus a finalize emitter
            # (softmax normalize on DVE + store on the sync ring).
            ps = ps_av.tile([128, QB], f32, tag="av", name=f"av{i}")

            def mm(j, ps=ps, i=i):
                nc.tensor.matmul(ps[:, 0:DV + 1],
                                 attnT[j][:, (i - j) * 128:(i - j) * 128 + 128],
                                 vaug[j][:, :], start=(j == 0), stop=(j == i))

            def fin(ps=ps, i=i):
                rec = out_p.tile([128, 1], f32, tag="rec")
                nc.vector.reciprocal(rec[:, :], ps[:, DV:DV + 1])
                ot = out_p.tile([128, DV], f32, tag="ot")
                nc.vector.tensor_scalar(ot[:, :], ps[:, 0:DV], rec[:, :], None,
                                        ALU.mult)
                nc.sync.dma_start(d_out[i * 128:(i + 1) * 128, :], ot[:, :])

            return [lambda j=j: mm(j) for j in range(i + 1)] + [fin]

        def av_group(g):
            ems = []
            for i in range(4 * g, 4 * g + 4):
                ems.extend(av_emitters(i))
            return ems

        def weave(pieces, fillers):
            # emit scores pieces with filler work spread between them so the
            # exp drain (one piece / ~560ns) never idles the PE; at most 2
            # fillers per piece so the piece matmuls keep the exp chain fed,
            # leftover fillers run while the last exps drain.
            na, nb = len(pieces), len(fillers)
            bi, acc = 0, 0.0
            r = nb / max(na, 1)
            for a in pieces:
                a()
                acc += r
                while acc >= 1.0 and bi < nb:
                    fillers[bi]()
                    bi += 1
                    acc -= 1.0
            while bi < nb:
                fillers[bi]()
                bi += 1

        def vtrans(j):
            ps = ps_p.tile([128, 128], bf16, tag="pp", name="vt")
            nc.tensor.transpose(ps[:, 0:128], vT[:, j * 128:(j + 1) * 128],
                                eye[:, :])
            va = attn_p.tile([128, DV + 1], bf16, tag=f"vaug{j}")
            nc.vector.tensor_copy(va[:, 0:DV], ps[:, 0:128])
            nc.vector.memset(va[:, DV:DV + 1], 1.0)
            vaug[j] = va

        def vstripe_emitters(g):
            ps = ps_p.tile([128, QB], f32, tag="pp", name=f"pp_v{g}")
            w = w_sb["wv"]

            def mm(c, ps=ps, g=g):
                nc.tensor.matmul(ps[:, :], w[:, c, :], xchunk("vT", g, c),
                                 start=(c == 0), stop=(c == EC - 1))

            def drain(ps=ps, g=g):
                nc.vector.tensor_scalar(vT[:, g * QB:(g + 1) * QB], ps[:, :],
                                        b_sb["bv"][:, :], None, ALU.add)

            return ([lambda c=c: mm(c) for c in range(EC)] + [drain] +
                    [lambda j=j: vtrans(j) for j in range(4 * g, 4 * g + 4)])

        # ---- front-loaded projections: everything whose data lands before
        # the scalar ring frees up (the exp-chain "wall" at ~30us) runs
        # first, so the post-wall segment only carries col0-col3 scores, the
        # AV chains, pair3 and the v2/v3 stripes — balancing the PE against
        # the ~22us exp chain instead of trailing it. ----
        for f in vstripe_emitters(0):
            f()
        proj_stripe("qT", 0, qT)
        proj_stripe("kT", 0, kT)
        for f in vstripe_emitters(1):
            f()
        proj_stripe("qT", 1, qT)
        proj_stripe("kT", 1, kT)
        proj_stripe("qT", 2, qT)
        proj_stripe("kT", 2, kT)

        def col(p):
            return [lambda j=j, p=p: scores_piece(j, p)
                    for j in range(4 * p + 4)]

        for a in col(0):
            a()
        weave(col(1), av_group(0))
        proj_stripe("qT", 3, qT)
        proj_stripe("kT", 3, kT)
        weave(col(2), av_group(1))
        av2 = av_group(2)
        weave(col(3), vstripe_emitters(2) + av2[:20]
              + vstripe_emitters(3) + av2[20:])

        # ---- last AV group ----
        for f in av_group(3):
            f()

    nc.compile()
    return nc


def _prep_inputs(pad_mask, query, key, value, Wq, bq, Wk, bk, Wv, bv):
    def wprep(w):
        return np.ascontiguousarray(
            np.asarray(w, np.float32).astype(_BF16).reshape(EC, 128, 128)
            .transpose(1, 0, 2))

    def bprep(v):
        return np.ascontiguousarray(np.asarray(v, np.float32).reshape(128, 1))

    shared = {
        "wq": wprep(Wq), "wk": wprep(Wk), "wv": wprep(Wv),
        "bq": bprep(bq), "bk": bprep(bk), "bv": bprep(bv),
        "tri": np.triu(np.ones((128, 128), np.float32)).astype(_BF16),
        "eye": np.eye(128, dtype=np.float32).astype(_BF16),
    }
    pad_mask = np.asarray(pad_mask)
    query = np.asarray(query, np.float32)
    key = np.asarray(key, np.float32)
    value = np.asarray(value, np.float32)

    def xprep(x):
        # x [S, E] -> per-ring stripe blocks [NG, 128, 4, QB]:
        # [g, p, r, s] = x[g*QB+s, h*512 + r*128 + p]
        a = x.reshape(NG, QB, 2, 4, 128)
        return (np.ascontiguousarray(a[:, :, 0].transpose(0, 3, 2, 1)).astype(_BF16),
                np.ascontiguousarray(a[:, :, 1].transpose(0, 3, 2, 1)).astype(_BF16))

    in_maps = []
    for b in range(B):
        padb = np.ascontiguousarray(
            np.where(pad_mask[b], NEG, np.float32(0.0)).reshape(SC, 128).T)
        qS, qC = xprep(query[b])
        kS, kC = xprep(key[b])
        vS, vC = xprep(value[b])
        in_maps.append({
            **shared,
            "qTS": qS, "qTC": qC, "kTS": kS, "kTC": kC, "vTS": vS, "vTC": vC,
            "padb": padb.astype(np.float32),
        })
    return in_maps


def _run(in_maps, trace=False, **kwargs):
    global _prog
    from concourse.bass_utils import run_bass_kernel_spmd
    if _prog is None:
        _prog = _build_program()
    return run_bass_kernel_spmd(_prog, in_maps, list(range(B)), trace=trace,
                                **kwargs)


def kernel(pad_mask, query, key, value, Wq, bq, Wk, bk, Wv, bv):
    in_maps = _prep_inputs(pad_mask, query, key, value, Wq, bq, Wk, bk, Wv, bv)
    res = _run(in_maps)
    out = np.stack([np.asarray(res.results[i]["out"]) for i in range(B)])
    return np.ascontiguousarray(out.astype(np.float32))
